# revision 1
# baseline (speedup 1.0000x reference)
"""Trainium2 Bass kernel for nn_DiffeqSolver (RK4 ODE solver with MLP vector field).

Reference computation (fp32):
    f(y) = tanh(tanh(y@W1 + b1) @ W2 + b2) @ W3 + b3
    RK4 fixed-step integration over T=50 time points, y: [TRAJ=4, B=256, D=256]
    output: [TRAJ, B, T, D]

Strategy:
  - Data parallel over 8 NeuronCores: flatten (TRAJ, B) -> 1024 rows, 128 rows
    per core. MLP weights replicated.
  - Whole RK4 scan runs on-chip; weights + state live in SBUF for all 49 steps.
  - "Transposed activation chain": all activations are stored feature-on-
    partition ([feat, row]), so every matmul is out[M=feat_chunk, N=rows] =
    W_tile[K, M].T @ actT[K, N] and no transposes are ever needed.
  - Matmul operands in fp16 (1 cycle/row on PE vs 4 for fp32), accumulation in
    fp32 PSUM, RK4 state and combines in fp32. Post-tanh activations are in
    [-1, 1] so fp16's 11-bit mantissa gives ~5e-4 per-eval relative error.
"""

import os
import sys
import time

sys.path.insert(0, "/opt/trn_rl_repo")

import numpy as np

TRAJ, B, D, H, T = 4, 256, 256, 1024, 50
NCORES = 8
R = TRAJ * B // NCORES  # 128 rows per core
DT2 = D // 128  # 2 d-chunks
HT = H // 128  # 8 h-chunks

_BUILD_CACHE = {}
LAST_RUN_SECONDS = None


def _mm_dt_str():
    return os.environ.get("DIFFEQ_MM_DT", "float16")


def _build_nc(n_steps, dts, mm_dt_str, zero_bias):
    """Build + finalize the Bacc program. dts: tuple of n_steps fp32 dt values."""
    import concourse.tile as tile
    from concourse import bacc, mybir

    f32 = mybir.dt.float32
    mm_dt = getattr(mybir.dt, mm_dt_str)
    Tanh = mybir.ActivationFunctionType.Identity  # placeholder, set below
    Tanh = mybir.ActivationFunctionType.Tanh
    Ident = mybir.ActivationFunctionType.Identity
    mult = mybir.AluOpType.mult
    add = mybir.AluOpType.add

    nc = bacc.Bacc("TRN2", target_bir_lowering=False, debug=False, num_devices=NCORES)

    y0_d = nc.declare_dram_parameter("y0", [128, D], f32, isOutput=False)
    w1_d = nc.declare_dram_parameter("w1", [D, H], mm_dt, isOutput=False)
    w2_d = nc.declare_dram_parameter("w2", [H, H], mm_dt, isOutput=False)
    w3_d = nc.declare_dram_parameter("w3", [H, D], mm_dt, isOutput=False)
    if not zero_bias:
        b1_d = nc.declare_dram_parameter("b1", [128, HT], f32, isOutput=False)
        b2_d = nc.declare_dram_parameter("b2", [128, HT], f32, isOutput=False)
        b3_d = nc.declare_dram_parameter("b3", [128, DT2], f32, isOutput=False)
    out_d = nc.declare_dram_parameter(
        "out", [n_steps + 1, DT2, 128, 128], f32, isOutput=True
    )

    with tile.TileContext(nc) as tc:
        with (
            tc.tile_pool(name="wp", bufs=1) as wp,
            tc.tile_pool(name="sp", bufs=2) as sp,
            tc.tile_pool(name="hp", bufs=2) as hp,
            tc.tile_pool(name="kp", bufs=2) as kp,
            tc.tile_pool(name="pp", bufs=1, space="PSUM") as pp,
        ):
            # --- persistent weights ---
            w1t = []
            for k in range(DT2):
                t_ = wp.tile([128, H], mm_dt, tag=f"w1_{k}")
                nc.gpsimd.dma_start(out=t_[:], in_=w1_d[128 * k : 128 * k + 128, :])
                w1t.append(t_)
            w2t = []
            for k in range(HT):
                t_ = wp.tile([128, H], mm_dt, tag=f"w2_{k}")
                nc.gpsimd.dma_start(out=t_[:], in_=w2_d[128 * k : 128 * k + 128, :])
                w2t.append(t_)
            w3t = []
            for k in range(HT):
                t_ = wp.tile([128, D], mm_dt, tag=f"w3_{k}")
                nc.gpsimd.dma_start(out=t_[:], in_=w3_d[128 * k : 128 * k + 128, :])
                w3t.append(t_)
            if not zero_bias:
                b1t = wp.tile([128, HT], f32, tag="b1")
                nc.gpsimd.dma_start(out=b1t[:], in_=b1_d[:])
                b2t = wp.tile([128, HT], f32, tag="b2")
                nc.gpsimd.dma_start(out=b2t[:], in_=b2_d[:])
                b3t = wp.tile([128, DT2], f32, tag="b3")
                nc.gpsimd.dma_start(out=b3t[:], in_=b3_d[:])

            # --- initial state ---
            y = sp.tile([128, D], f32, tag="y")
            nc.gpsimd.dma_start(out=y[:], in_=y0_d[:])
            for j in range(DT2):
                nc.gpsimd.dma_start(
                    out=out_d[0, j], in_=y[:, 128 * j : 128 * j + 128]
                )
            yh = sp.tile([128, D], mm_dt, tag="yh")
            nc.scalar.copy(yh[:], y[:])

            def eval_f(xh, ev):
                """xh: [128, D] mm_dt tile (transposed input). Returns k tile
                [128, D] fp32 (= f(x), transposed layout)."""
                # layer 1: D -> H, tanh
                h1 = []
                for half in range(2):
                    ps = pp.tile([128, 512], f32, tag=f"p1{half}")
                    for mi in range(4):
                        m = 4 * half + mi
                        for k in range(DT2):
                            nc.tensor.matmul(
                                ps[:, 128 * mi : 128 * mi + 128],
                                w1t[k][:, 128 * m : 128 * m + 128],
                                xh[:, 128 * k : 128 * k + 128],
                                start=(k == 0),
                                stop=(k == DT2 - 1),
                            )
                    ht = hp.tile([128, 512], mm_dt, tag=f"h1_{half}")
                    if zero_bias:
                        nc.scalar.activation(ht[:], ps[:], Tanh)
                    else:
                        for mi in range(4):
                            m = 4 * half + mi
                            nc.scalar.activation(
                                ht[:, 128 * mi : 128 * mi + 128],
                                ps[:, 128 * mi : 128 * mi + 128],
                                Tanh,
                                bias=b1t[:, m : m + 1],
                            )
                    h1.append(ht)

                # layer 2: H -> H, tanh
                h2 = []
                for half in range(2):
                    ps = pp.tile([128, 512], f32, tag=f"p2{half}")
                    for mi in range(4):
                        m = 4 * half + mi
                        for k in range(HT):
                            nc.tensor.matmul(
                                ps[:, 128 * mi : 128 * mi + 128],
                                w2t[k][:, 128 * m : 128 * m + 128],
                                h1[k // 4][:, 128 * (k % 4) : 128 * (k % 4) + 128],
                                start=(k == 0),
                                stop=(k == HT - 1),
                            )
                    ht = hp.tile([128, 512], mm_dt, tag=f"h2_{half}")
                    if zero_bias:
                        nc.scalar.activation(ht[:], ps[:], Tanh)
                    else:
                        for mi in range(4):
                            m = 4 * half + mi
                            nc.scalar.activation(
                                ht[:, 128 * mi : 128 * mi + 128],
                                ps[:, 128 * mi : 128 * mi + 128],
                                Tanh,
                                bias=b2t[:, m : m + 1],
                            )
                    h2.append(ht)

                # layer 3: H -> D (no tanh)
                ps = pp.tile([128, D], f32, tag="p3")
                for j in range(DT2):
                    for k in range(HT):
                        nc.tensor.matmul(
                            ps[:, 128 * j : 128 * j + 128],
                            w3t[k][:, 128 * j : 128 * j + 128],
                            h2[k // 4][:, 128 * (k % 4) : 128 * (k % 4) + 128],
                            start=(k == 0),
                            stop=(k == HT - 1),
                        )
                kt = kp.tile([128, D], f32, tag=f"k{ev}")
                if zero_bias:
                    nc.vector.tensor_copy(kt[:], ps[:])
                else:
                    for j in range(DT2):
                        nc.scalar.activation(
                            kt[:, 128 * j : 128 * j + 128],
                            ps[:, 128 * j : 128 * j + 128],
                            Ident,
                            bias=b3t[:, j : j + 1],
                        )
                return kt

            for t in range(1, n_steps + 1):
                dt = float(dts[t - 1])
                half_dt = float(np.float32(0.5) * np.float32(dt))
                dt6 = float(np.float32(dt) / np.float32(6.0))

                k1 = eval_f(yh, 1)
                ya = sp.tile([128, D], mm_dt, tag="ya")
                nc.vector.scalar_tensor_tensor(ya[:], k1[:], half_dt, y[:], mult, add)
                k2 = eval_f(ya, 2)
                yb = sp.tile([128, D], mm_dt, tag="yb")
                nc.vector.scalar_tensor_tensor(yb[:], k2[:], half_dt, y[:], mult, add)
                k3 = eval_f(yb, 3)
                yc = sp.tile([128, D], mm_dt, tag="yc")
                nc.vector.scalar_tensor_tensor(yc[:], k3[:], dt, y[:], mult, add)
                k4 = eval_f(yc, 4)

                s1 = kp.tile([128, D], f32, tag="s1")
                nc.vector.tensor_tensor(s1[:], k2[:], k3[:], add)
                s2 = kp.tile([128, D], f32, tag="s2")
                nc.vector.tensor_tensor(s2[:], k1[:], k4[:], add)
                acc = kp.tile([128, D], f32, tag="acc")
                nc.vector.scalar_tensor_tensor(acc[:], s1[:], 2.0, s2[:], mult, add)
                ynew = sp.tile([128, D], f32, tag="y")
                nc.vector.scalar_tensor_tensor(ynew[:], acc[:], dt6, y[:], mult, add)
                y = ynew
                if t < n_steps:
                    yh = sp.tile([128, D], mm_dt, tag="yh")
                    nc.scalar.copy(yh[:], y[:])
                for j in range(DT2):
                    nc.gpsimd.dma_start(
                        out=out_d[t, j], in_=y[:, 128 * j : 128 * j + 128]
                    )

    nc.finalize()
    return nc


def _get_nc(n_steps, dts, mm_dt_str, zero_bias):
    key = (n_steps, dts, mm_dt_str, zero_bias)
    if key not in _BUILD_CACHE:
        _BUILD_CACHE[key] = _build_nc(n_steps, dts, mm_dt_str, zero_bias)
    return _BUILD_CACHE[key]


def _enable_jax_cache():
    try:
        import jax

        jax.config.update("jax_compilation_cache_dir", "/tmp/jax_diffeq_cache")
        jax.config.update("jax_persistent_cache_min_compile_time_secs", 1.0)
    except Exception:
        pass


def kernel(
    first_point,
    time_steps_to_predict,
    W1,
    b1,
    W2,
    b2,
    W3,
    b3,
):
    global LAST_RUN_SECONDS
    _enable_jax_cache()
    from concourse.bass_utils import run_bass_kernel_spmd

    first_point = np.asarray(first_point)
    ts = np.asarray(time_steps_to_predict, dtype=np.float32)
    n_steps = int(ts.shape[0]) - 1
    n_steps_override = os.environ.get("DIFFEQ_NSTEPS")
    if n_steps_override is not None:
        n_steps = int(n_steps_override)
    dts = tuple(float(x) for x in (ts[1:] - ts[:-1])[:n_steps])
    mm_dt_str = _mm_dt_str()

    W1 = np.asarray(W1, dtype=np.float32)
    W2 = np.asarray(W2, dtype=np.float32)
    W3 = np.asarray(W3, dtype=np.float32)
    b1 = np.asarray(b1, dtype=np.float32)
    b2 = np.asarray(b2, dtype=np.float32)
    b3 = np.asarray(b3, dtype=np.float32)
    zero_bias = not (np.any(b1) or np.any(b2) or np.any(b3))

    nc = _get_nc(n_steps, dts, mm_dt_str, zero_bias)

    np_mm_dt = np.float16 if mm_dt_str == "float16" else np.float32
    w1h = np.ascontiguousarray(W1.astype(np_mm_dt))
    w2h = np.ascontiguousarray(W2.astype(np_mm_dt))
    w3h = np.ascontiguousarray(W3.astype(np_mm_dt))

    fp = first_point.astype(np.float32).reshape(TRAJ * B, D)
    in_maps = []
    for c in range(NCORES):
        shard = fp[c * R : (c + 1) * R]  # [128 rows, 256 feat]
        # y0 tile layout: [128 partitions, 2*128 free]; partition p of free
        # slice j holds feature 128j+p over rows -> y0[p, 128j+r] = shard[r, 128j+p]
        y0 = np.ascontiguousarray(
            shard.T.reshape(DT2, 128, R).transpose(1, 0, 2).reshape(128, DT2 * R)
        )
        m = {"y0": y0, "w1": w1h, "w2": w2h, "w3": w3h}
        if not zero_bias:
            m["b1"] = np.ascontiguousarray(b1.reshape(HT, 128).T)
            m["b2"] = np.ascontiguousarray(b2.reshape(HT, 128).T)
            m["b3"] = np.ascontiguousarray(b3.reshape(DT2, 128).T)
        in_maps.append(m)

    t0 = time.time()
    res = run_bass_kernel_spmd(nc, in_maps, list(range(NCORES)))
    LAST_RUN_SECONDS = time.time() - t0

    # assemble: per-core out [n_steps+1, DT2, 128, 128] (t, j, p, r) where
    # feature d = 128j+p -> want [R rows, T, D]
    shards = []
    for c in range(NCORES):
        oc = res.results[c]["out"]  # [S, 2, 128, 128]
        shards.append(np.transpose(oc, (3, 0, 1, 2)).reshape(R, n_steps + 1, D))
    full = np.concatenate(shards, axis=0)  # [1024, S, 256]
    if n_steps + 1 < T:
        pad = np.zeros((TRAJ * B, T - (n_steps + 1), D), np.float32)
        full = np.concatenate([full, pad], axis=1)
    return np.ascontiguousarray(full.reshape(TRAJ, B, T, D))


# revision 3
# speedup vs baseline: 2.3415x; 2.3415x over previous
"""Trainium2 Bass kernel for nn_DiffeqSolver (RK4 ODE solver with MLP vector field).

Reference computation (fp32):
    f(y) = tanh(tanh(y@W1 + b1) @ W2 + b2) @ W3 + b3
    RK4 fixed-step integration over T=50 time points, y: [TRAJ=4, B=256, D=256]
    output: [TRAJ, B, T, D]

Strategy:
  - Data parallel over 8 NeuronCores: flatten (TRAJ, B) -> 1024 rows, 128 rows
    per core. MLP weights replicated, whole RK4 scan on-chip (weights + state
    stay in SBUF for all 49 steps).
  - "Transposed activation chain": activations stored feature-on-partition
    ([feat, row]); every matmul is out[M=feat_chunk, N=rows] = W[K, M].T @
    actT[K, N], so no transposes are ever needed.
  - Matmul operands fp16 (1 cycle/row on PE vs 4 for fp32), fp32 PSUM
    accumulation, fp32 state/combines. Post-tanh activations are in [-1, 1]
    so fp16 costs ~5e-4 relative per f-eval; measured end-to-end rel err
    ~2e-4 over 49 steps.
  - Latency hiding: k-outer matmul emission (PE starts each layer as soon as
    the first input chunk exists), fine-grained L3 PSUM (next eval's layer 1
    starts while this eval's layer 3 finishes), RK4 stage inputs produced by
    single fused DVE ops reading PSUM directly.
"""

import os
import sys
import time

sys.path.insert(0, "/opt/trn_rl_repo")

import numpy as np

TRAJ, B, D, H, T = 4, 256, 256, 1024, 50
NCORES = 8
R = TRAJ * B // NCORES  # 128 rows per core
DT2 = D // 128  # 2 d-chunks
HT = H // 128  # 8 h-chunks

_BUILD_CACHE = {}
LAST_RUN_SECONDS = None


def _mm_dt_str():
    return os.environ.get("DIFFEQ_MM_DT", "float16")


def _build_nc(n_steps, dts, mm_dt_str, zero_bias):
    """Build + finalize the Bacc program. dts: tuple of n_steps fp32 dt values."""
    import concourse.tile as tile
    from concourse import bacc, mybir

    f32 = mybir.dt.float32
    mm_dt = getattr(mybir.dt, mm_dt_str)
    Tanh = mybir.ActivationFunctionType.Tanh
    Ident = mybir.ActivationFunctionType.Identity
    mult = mybir.AluOpType.mult
    add = mybir.AluOpType.add

    nc = bacc.Bacc("TRN2", target_bir_lowering=False, debug=False, num_devices=NCORES)

    y0_d = nc.declare_dram_parameter("y0", [128, D], f32, isOutput=False)
    w1_d = nc.declare_dram_parameter("w1", [D, H], mm_dt, isOutput=False)
    w2_d = nc.declare_dram_parameter("w2", [H, H], mm_dt, isOutput=False)
    w3_d = nc.declare_dram_parameter("w3", [H, D], mm_dt, isOutput=False)
    if not zero_bias:
        b1_d = nc.declare_dram_parameter("b1", [128, HT], f32, isOutput=False)
        b2_d = nc.declare_dram_parameter("b2", [128, HT], f32, isOutput=False)
        b3_d = nc.declare_dram_parameter("b3", [128, DT2], f32, isOutput=False)
    out_d = nc.declare_dram_parameter(
        "out", [n_steps + 1, DT2, 128, 128], f32, isOutput=True
    )

    with tile.TileContext(nc) as tc:
        with (
            tc.tile_pool(name="wp", bufs=1) as wp,
            tc.tile_pool(name="sp", bufs=2) as sp,
            tc.tile_pool(name="hp", bufs=2) as hp,
            tc.tile_pool(name="kp", bufs=2) as kp,
            tc.tile_pool(name="pp", bufs=1, space="PSUM") as pp,
        ):
            # --- persistent weights ---
            w1t = []
            for k in range(DT2):
                t_ = wp.tile([128, H], mm_dt, tag=f"w1_{k}")
                nc.gpsimd.dma_start(out=t_[:], in_=w1_d[128 * k : 128 * k + 128, :])
                w1t.append(t_)
            w2t = []
            for k in range(HT):
                t_ = wp.tile([128, H], mm_dt, tag=f"w2_{k}")
                nc.gpsimd.dma_start(out=t_[:], in_=w2_d[128 * k : 128 * k + 128, :])
                w2t.append(t_)
            w3t = []
            for k in range(HT):
                t_ = wp.tile([128, D], mm_dt, tag=f"w3_{k}")
                nc.gpsimd.dma_start(out=t_[:], in_=w3_d[128 * k : 128 * k + 128, :])
                w3t.append(t_)
            if not zero_bias:
                b1t = wp.tile([128, HT], f32, tag="b1")
                nc.gpsimd.dma_start(out=b1t[:], in_=b1_d[:])
                b2t = wp.tile([128, HT], f32, tag="b2")
                nc.gpsimd.dma_start(out=b2t[:], in_=b2_d[:])
                b3t = wp.tile([128, DT2], f32, tag="b3")
                nc.gpsimd.dma_start(out=b3t[:], in_=b3_d[:])

            # --- initial state ---
            y = sp.tile([128, D], f32, tag="y")
            nc.gpsimd.dma_start(out=y[:], in_=y0_d[:])
            for j in range(DT2):
                nc.gpsimd.dma_start(out=out_d[0, j], in_=y[:, 128 * j : 128 * j + 128])
            yh = sp.tile([128, D], mm_dt, tag="yh")
            nc.scalar.copy(yh[:], y[:])

            def eval_f(xh, ev):
                """xh: [128, D] mm_dt tile (transposed input). Returns list of
                DT2 PSUM tiles [128, 128] fp32 holding f(x) pre-bias (chunk j),
                i.e. the caller reads them (b3 handled by caller paths)."""
                # ---- layer 1: D -> H, tanh; 2 psum banks of [128, 512]
                ps1 = [pp.tile([128, 512], f32, tag=f"p1{h}", name=f"ps1_{h}") for h in range(2)]
                for k in range(DT2):
                    for m in range(HT):
                        nc.tensor.matmul(
                            ps1[m // 4][:, 128 * (m % 4) : 128 * (m % 4) + 128],
                            w1t[k][:, 128 * m : 128 * m + 128],
                            xh[:, 128 * k : 128 * k + 128],
                            start=(k == 0),
                            stop=(k == DT2 - 1),
                        )
                h1 = []
                for h in range(2):
                    ht = hp.tile([128, 512], mm_dt, tag=f"h1_{h}")
                    if zero_bias:
                        nc.scalar.activation(ht[:], ps1[h][:], Tanh)
                    else:
                        for mi in range(4):
                            m = 4 * h + mi
                            nc.scalar.activation(
                                ht[:, 128 * mi : 128 * mi + 128],
                                ps1[h][:, 128 * mi : 128 * mi + 128],
                                Tanh,
                                bias=b1t[:, m : m + 1],
                            )
                    h1.append(ht)

                # ---- layer 2: H -> H, tanh; 4 psum banks of [128, 256]
                ps2 = [pp.tile([128, 256], f32, tag=f"p2{q}", name=f"ps2_{q}") for q in range(4)]
                for k in range(HT):
                    rhs = h1[k // 4][:, 128 * (k % 4) : 128 * (k % 4) + 128]
                    for m in range(HT):
                        nc.tensor.matmul(
                            ps2[m // 2][:, 128 * (m % 2) : 128 * (m % 2) + 128],
                            w2t[k][:, 128 * m : 128 * m + 128],
                            rhs,
                            start=(k == 0),
                            stop=(k == HT - 1),
                        )
                h2 = []
                for q in range(4):
                    ht = hp.tile([128, 256], mm_dt, tag=f"h2_{q}")
                    if zero_bias:
                        nc.scalar.activation(ht[:], ps2[q][:], Tanh)
                    else:
                        for mi in range(2):
                            m = 2 * q + mi
                            nc.scalar.activation(
                                ht[:, 128 * mi : 128 * mi + 128],
                                ps2[q][:, 128 * mi : 128 * mi + 128],
                                Tanh,
                                bias=b2t[:, m : m + 1],
                            )
                    h2.append(ht)

                # ---- layer 3: H -> D, no tanh; 2 psum banks of [128, 128]
                ps3 = [pp.tile([128, 128], f32, tag=f"p3{j}", name=f"ps3_{j}") for j in range(DT2)]
                for k in range(HT):
                    rhs = h2[k // 2][:, 128 * (k % 2) : 128 * (k % 2) + 128]
                    for j in range(DT2):
                        nc.tensor.matmul(
                            ps3[j][:],
                            w3t[k][:, 128 * j : 128 * j + 128],
                            rhs,
                            start=(k == 0),
                            stop=(k == HT - 1),
                        )
                return ps3

            def k_from_psum(ps3, ev):
                """Copy f(x) out of PSUM into an SBUF fp32 tile (adding b3 when
                nonzero). Off the critical path for k1..k3."""
                kt = kp.tile([128, D], f32, tag=f"k{ev}")
                for j in range(DT2):
                    if zero_bias:
                        nc.vector.tensor_copy(kt[:, 128 * j : 128 * j + 128], ps3[j][:])
                    else:
                        nc.scalar.activation(
                            kt[:, 128 * j : 128 * j + 128],
                            ps3[j][:],
                            Ident,
                            bias=b3t[:, j : j + 1],
                        )
                return kt

            def stage_input(ps3, coef, y, tag):
                """x_stage = coef * f + y, written per chunk directly from PSUM
                (zero-bias path) so the next eval starts after chunk 0."""
                st = sp.tile([128, D], mm_dt, tag=tag)
                for j in range(DT2):
                    sl = slice(128 * j, 128 * j + 128)
                    nc.vector.scalar_tensor_tensor(
                        st[:, sl], ps3[j][:], coef, y[:, sl], mult, add
                    )
                return st

            for t in range(1, n_steps + 1):
                dt = float(dts[t - 1])
                half_dt = float(np.float32(0.5) * np.float32(dt))
                dt6 = float(np.float32(dt) / np.float32(6.0))

                if zero_bias:
                    p_k1 = eval_f(yh, 1)
                    ya = stage_input(p_k1, half_dt, y, "ya")
                    k1 = k_from_psum(p_k1, 1)
                    p_k2 = eval_f(ya, 2)
                    yb = stage_input(p_k2, half_dt, y, "yb")
                    k2 = k_from_psum(p_k2, 2)
                    p_k3 = eval_f(yb, 3)
                    yc = stage_input(p_k3, dt, y, "yc")
                    k3 = k_from_psum(p_k3, 3)
                    p_k4 = eval_f(yc, 4)

                    s1 = kp.tile([128, D], f32, tag="s1")
                    nc.vector.tensor_tensor(s1[:], k2[:], k3[:], add)
                    s2 = kp.tile([128, D], f32, tag="s2")
                    for j in range(DT2):
                        sl = slice(128 * j, 128 * j + 128)
                        nc.vector.tensor_tensor(s2[:, sl], k1[:, sl], p_k4[j][:], add)
                    acc = kp.tile([128, D], f32, tag="acc")
                    nc.vector.scalar_tensor_tensor(acc[:], s1[:], 2.0, s2[:], mult, add)
                else:
                    p1_ = eval_f(yh, 1)
                    k1 = k_from_psum(p1_, 1)
                    ya = sp.tile([128, D], mm_dt, tag="ya")
                    nc.vector.scalar_tensor_tensor(ya[:], k1[:], half_dt, y[:], mult, add)
                    p2_ = eval_f(ya, 2)
                    k2 = k_from_psum(p2_, 2)
                    yb = sp.tile([128, D], mm_dt, tag="yb")
                    nc.vector.scalar_tensor_tensor(yb[:], k2[:], half_dt, y[:], mult, add)
                    p3_ = eval_f(yb, 3)
                    k3 = k_from_psum(p3_, 3)
                    yc = sp.tile([128, D], mm_dt, tag="yc")
                    nc.vector.scalar_tensor_tensor(yc[:], k3[:], dt, y[:], mult, add)
                    p4_ = eval_f(yc, 4)
                    k4 = k_from_psum(p4_, 4)
                    s1 = kp.tile([128, D], f32, tag="s1")
                    nc.vector.tensor_tensor(s1[:], k2[:], k3[:], add)
                    s2 = kp.tile([128, D], f32, tag="s2")
                    nc.vector.tensor_tensor(s2[:], k1[:], k4[:], add)
                    acc = kp.tile([128, D], f32, tag="acc")
                    nc.vector.scalar_tensor_tensor(acc[:], s1[:], 2.0, s2[:], mult, add)

                # y' = y + dt/6 * acc, produced twice: fp16 copy feeds the next
                # step's first eval immediately; fp32 copy is the state.
                ynew = sp.tile([128, D], f32, tag="y")
                if t < n_steps:
                    yh = sp.tile([128, D], mm_dt, tag="yh")
                    nc.vector.scalar_tensor_tensor(yh[:], acc[:], dt6, y[:], mult, add)
                nc.vector.scalar_tensor_tensor(ynew[:], acc[:], dt6, y[:], mult, add)
                y = ynew
                for j in range(DT2):
                    nc.gpsimd.dma_start(
                        out=out_d[t, j], in_=y[:, 128 * j : 128 * j + 128]
                    )

    nc.finalize()
    return nc


def _get_nc(n_steps, dts, mm_dt_str, zero_bias):
    key = (n_steps, dts, mm_dt_str, zero_bias)
    if key not in _BUILD_CACHE:
        _BUILD_CACHE[key] = _build_nc(n_steps, dts, mm_dt_str, zero_bias)
    return _BUILD_CACHE[key]


def _enable_jax_cache():
    try:
        import jax

        jax.config.update("jax_compilation_cache_dir", "/tmp/jax_diffeq_cache")
        jax.config.update("jax_persistent_cache_min_compile_time_secs", 1.0)
    except Exception:
        pass


def kernel(
    first_point,
    time_steps_to_predict,
    W1,
    b1,
    W2,
    b2,
    W3,
    b3,
):
    global LAST_RUN_SECONDS
    _enable_jax_cache()
    from concourse.bass_utils import run_bass_kernel_spmd

    first_point = np.asarray(first_point)
    ts = np.asarray(time_steps_to_predict, dtype=np.float32)
    n_steps = int(ts.shape[0]) - 1
    n_steps_override = os.environ.get("DIFFEQ_NSTEPS")
    if n_steps_override is not None:
        n_steps = int(n_steps_override)
    dts = tuple(float(x) for x in (ts[1:] - ts[:-1])[:n_steps])
    mm_dt_str = _mm_dt_str()

    W1 = np.asarray(W1, dtype=np.float32)
    W2 = np.asarray(W2, dtype=np.float32)
    W3 = np.asarray(W3, dtype=np.float32)
    b1 = np.asarray(b1, dtype=np.float32)
    b2 = np.asarray(b2, dtype=np.float32)
    b3 = np.asarray(b3, dtype=np.float32)
    zero_bias = not (np.any(b1) or np.any(b2) or np.any(b3))

    nc = _get_nc(n_steps, dts, mm_dt_str, zero_bias)

    np_mm_dt = np.float16 if mm_dt_str == "float16" else np.float32
    w1h = np.ascontiguousarray(W1.astype(np_mm_dt))
    w2h = np.ascontiguousarray(W2.astype(np_mm_dt))
    w3h = np.ascontiguousarray(W3.astype(np_mm_dt))

    fp = first_point.astype(np.float32).reshape(TRAJ * B, D)
    in_maps = []
    for c in range(NCORES):
        shard = fp[c * R : (c + 1) * R]  # [128 rows, 256 feat]
        # y0 tile layout: [128 partitions, 2*128 free]; partition p of free
        # slice j holds feature 128j+p over rows -> y0[p, 128j+r] = shard[r, 128j+p]
        y0 = np.ascontiguousarray(
            shard.T.reshape(DT2, 128, R).transpose(1, 0, 2).reshape(128, DT2 * R)
        )
        m = {"y0": y0, "w1": w1h, "w2": w2h, "w3": w3h}
        if not zero_bias:
            m["b1"] = np.ascontiguousarray(b1.reshape(HT, 128).T)
            m["b2"] = np.ascontiguousarray(b2.reshape(HT, 128).T)
            m["b3"] = np.ascontiguousarray(b3.reshape(DT2, 128).T)
        in_maps.append(m)

    t0 = time.time()
    res = run_bass_kernel_spmd(nc, in_maps, list(range(NCORES)))
    LAST_RUN_SECONDS = time.time() - t0

    # assemble: per-core out [n_steps+1, DT2, 128, 128] (t, j, p, r) where
    # feature d = 128j+p -> want [R rows, T, D]
    shards = []
    for c in range(NCORES):
        oc = res.results[c]["out"]  # [S, 2, 128, 128]
        shards.append(np.transpose(oc, (3, 0, 1, 2)).reshape(R, n_steps + 1, D))
    full = np.concatenate(shards, axis=0)  # [1024, S, 256]
    if n_steps + 1 < T:
        pad = np.zeros((TRAJ * B, T - (n_steps + 1), D), np.float32)
        full = np.concatenate([full, pad], axis=1)
    return np.ascontiguousarray(full.reshape(TRAJ, B, T, D))
